# revision 27
# baseline (speedup 1.0000x reference)
"""Trainium2 Bass kernel for nn_Encoder_trace (GNN message passing + cross-attention).

Data-parallel over the batch axis B=64 across 8 NeuronCores (8 graphs/core).
Device layout: channels on SBUF partitions, tokens on the free dimension
(everything computed transposed; host un-transposes on gather).

v2: all weight-combination products are precomputed on the HOST (numpy), so
the device kernel is just the per-graph streaming work.  All matmul operands
are bf16.  The chain-GCN aggregation is applied once to xT (it commutes with
every row-mix), the softmax denominators are produced by a ones-column folded
into the vvh stationary, and the reciprocal rows are broadcast across
partitions with a tiny [2,128] selection matmul.

Math per graph (g):
  xa      = agg(xT)                                  (DVE fixup on cols 0..4)
  x_timeT = W_comb @ xa + bxtf                       (W_comb = W_gcn W_lin)
  qT      = W_qcomb @ xa + bqc                       (W_qcomb = Wq W_gcn W_lin)
  kT      = Wk @ WE + bk                 [host]
  vvh     = WE.T @ Wv.T (ones col interleaved per head)  [host]
  per head: scoresT = kT_h.T @ qT_h ; exp (ACT, scale=1/8, no max-sub)
            op[0:65] = [vvh_h | 1].T @ exp           (row 64 = softmax sums)
            rc = recip(op[64]) ; rcb = sel-matmul broadcast of rc
            oT = op[0:64] * rcb
  x_outT  = W_out @ oT + boute                       (boute = W_out bv + b_out)
"""

import numpy as np
from contextlib import ExitStack

import concourse.bass as bass
import concourse.mybir as mybir
import concourse.tile as tile
from concourse.bass import ts, ds

# problem dims (hardcoded per spec)
B, F, D, H, NH, DH, V = 64, 512, 256, 768, 12, 64, 256
NCORES = 8
G = B // NCORES       # graphs per core
KH = H // 128         # 6  (H in 128-partition tiles)
KD = D // 128         # 2  (D in 128-partition tiles)
NPAIR = NH // 2       # 6  head pairs

F32 = mybir.dt.float32
BF16 = mybir.dt.bfloat16
AF = mybir.ActivationFunctionType
ALU = mybir.AluOpType

WT = BF16
WT_NP = mybir.dt.np(WT)

RSQRT2 = float(2.0 ** -0.5)
C1 = RSQRT2 - 0.5  # chain-GCN col-1 colsum deviation (bias correction coeff)


def build_program(bias_fix=False):
    nc = bass.Bass()

    xt_d = nc.declare_dram_parameter("xt", [G, D, F], WT, isOutput=False)
    wqc_d = nc.declare_dram_parameter("w_qcomb", [D, H], WT, isOutput=False)
    wcb_d = nc.declare_dram_parameter("w_comb", [D, H], WT, isOutput=False)
    wot_d = nc.declare_dram_parameter("w_out_t", [H, H], WT, isOutput=False)
    kt_d = nc.declare_dram_parameter("k_t", [H, V], WT, isOutput=False)
    vva_d = nc.declare_dram_parameter("vvh_a", [D, NH * (DH + 1)], WT, isOutput=False)
    bqc_d = nc.declare_dram_parameter("b_qc", [H, 1], F32, isOutput=False)
    bxt_d = nc.declare_dram_parameter("b_xt", [H, 1], F32, isOutput=False)
    bou_d = nc.declare_dram_parameter("b_oute", [H, 1], F32, isOutput=False)
    ones_d = nc.declare_dram_parameter("ones_r", [1, DH], WT, isOutput=False)
    if bias_fix:
        cb1_d = nc.declare_dram_parameter("c_b1", [H, 1], F32, isOutput=False)
        cq1_d = nc.declare_dram_parameter("c_q1", [H, 1], F32, isOutput=False)
    oxt_d = nc.declare_dram_parameter("out_xt", [G, H, F], BF16, isOutput=True)
    oxo_d = nc.declare_dram_parameter("out_xo", [G, H, F], BF16, isOutput=True)

    with ExitStack() as ctx:
        tc = ctx.enter_context(tile.TileContext(nc))
        wp = ctx.enter_context(tc.tile_pool(name="wp", bufs=1))
        pp = ctx.enter_context(tc.tile_pool(name="pp", bufs=1, space="PSUM"))
        dp = ctx.enter_context(tc.tile_pool(name="dp", bufs=1))

        def ptile(shape, tag, bufs):
            return pp.tile(shape, F32, name=tag, tag=tag, bufs=bufs)

        def wtile(shape, dt, tag):
            return wp.tile(shape, dt, name=tag, tag=tag)


        # ---------------- persistent weights (DMA in consumer order) -------
        wqc = [wtile([128, H], WT, f"wqc{k}") for k in range(KD)]
        wcb = [wtile([128, H], WT, f"wcb{k}") for k in range(KD)]
        kt = [wtile([128, V], WT, f"kt{m}") for m in range(KH)]
        vva = [wtile([128, NH * (DH + 1)], WT, f"vva{m}") for m in range(KD)]
        wout = [wtile([128, H], WT, f"wout{k}") for k in range(KH)]
        onesb = wtile([1, DH], WT, "onesb")
        bqc = [wtile([128, 1], F32, f"bqc{m}") for m in range(KH)]
        bxtf = [wtile([128, 1], F32, f"bxtf{m}") for m in range(KH)]
        boute = [wtile([128, 1], F32, f"boute{m}") for m in range(KH)]
        if bias_fix:
            cb1 = [wtile([128, 1], F32, f"cb1{m}") for m in range(KH)]
            cq1 = [wtile([128, 1], F32, f"cq1{m}") for m in range(KH)]

        for k in range(KD):
            nc.gpsimd.dma_start(wqc[k][:, :], wqc_d[ts(k, 128), :])
        for k in range(KD):
            nc.gpsimd.dma_start(wcb[k][:, :], wcb_d[ts(k, 128), :])
        for m in range(KH):
            nc.gpsimd.dma_start(bqc[m][:, :], bqc_d[ts(m, 128), :])
            nc.gpsimd.dma_start(bxtf[m][:, :], bxt_d[ts(m, 128), :])
            if bias_fix:
                nc.gpsimd.dma_start(cb1[m][:, :], cb1_d[ts(m, 128), :])
                nc.gpsimd.dma_start(cq1[m][:, :], cq1_d[ts(m, 128), :])
        for m in range(KH):
            nc.gpsimd.dma_start(kt[m][:, :], kt_d[ts(m, 128), :])
        for m in range(KD):
            nc.gpsimd.dma_start(vva[m][:, :], vva_d[ts(m, 128), :])
        nc.gpsimd.dma_start(onesb[:, :], ones_d[:, :])
        for k in range(KH):
            nc.gpsimd.dma_start(wout[k][:, :], wot_d[ts(k, 128), :])
        for m in range(KH):
            nc.gpsimd.dma_start(boute[m][:, :], bou_d[ts(m, 128), :])

        # ---------------- per-graph emission ----------------
        def emit_xt_dma(g):
            xts = []
            for k in range(KD):
                t = dp.tile([128, F], WT, name="xtin", tag="xtin", bufs=4)
                nc.sync.dma_start(t[:, :], xt_d[g, ts(k, 128), :])
                xts.append(t)
            return xts

        def emit_fixups(g, xts):
            # chain-GCN aggregation applied once to xT (commutes with the
            # row mixes).  Cols: 0: c0; 1: .5c1+r2*c0; 2..4: .5ck+.5c(k-1).
            for t in xts:
                xs = dp.tile([128, 4], WT, name="xsc", tag="xsc", bufs=4)
                nc.vector.tensor_copy(xs[:, :], t[:, 0:4])
                nc.vector.tensor_scalar_mul(t[:, 1:5], t[:, 1:5], 0.5)
                nc.vector.scalar_tensor_tensor(
                    t[:, 2:5], xs[:, 1:4], 0.5, t[:, 2:5], ALU.mult, ALU.add
                )
                nc.vector.scalar_tensor_tensor(
                    t[:, 1:2], xs[:, 0:1], RSQRT2, t[:, 1:2], ALU.mult, ALU.add
                )

        def emit_front_q(g, xts, m):
            ps = ptile([128, F], "op", 4)
            for k in range(KD):
                nc.tensor.matmul(
                    ps[:, :], wqc[k][:, ts(m, 128)], xts[k][:, :],
                    start=(k == 0), stop=(k == KD - 1),
                )
            qt = dp.tile([128, F], WT, name="qt", tag="qt", bufs=12)
            nc.scalar.activation(
                qt[:, :], ps[:, :], AF.Identity, bias=bqc[m][:, :], scale=1.0
            )
            if bias_fix:
                nc.vector.scalar_tensor_tensor(
                    qt[:, 1:2], cq1[m][:, :], 1.0, qt[:, 1:2], ALU.mult, ALU.add
                )
            return qt

        def emit_front_h(g, xts, m):
            ps = ptile([128, F], "op", 4)
            for k in range(KD):
                nc.tensor.matmul(
                    ps[:, :], wcb[k][:, ts(m, 128)], xts[k][:, :],
                    start=(k == 0), stop=(k == KD - 1),
                )
            xo = dp.tile([128, F], BF16, name="xtime", tag="xtime", bufs=3)
            nc.scalar.activation(
                xo[:, :], ps[:, :], AF.Identity, bias=bxtf[m][:, :], scale=1.0
            )
            if bias_fix:
                nc.vector.scalar_tensor_tensor(
                    xo[:, 1:2], cb1[m][:, :], 1.0, xo[:, 1:2], ALU.mult, ALU.add
                )
            nc.gpsimd.dma_start(oxt_d[g, ts(m, 128), :], xo[:, :])

        def emit_scores(j, qts):
            exps = []
            for hh in range(2):
                r = DH * hh
                sc = ptile([128, 2 * F], "score", 2)
                for vh in range(2):
                    nc.tensor.matmul(
                        sc[:, ts(vh, F)],
                        kt[j][r : r + DH, ts(vh, 128)],
                        qts[j][r : r + DH, :],
                        start=True, stop=True,
                    )
                ex = dp.tile([128, 2 * F], WT, name="exp", tag="exp", bufs=4)
                nc.scalar.activation(ex[:, :], sc[:, :], AF.Exp, scale=0.125)
                exps.append(ex)
            return exps

        def emit_tail_a(j, exps):
            ops, rcs = [], []
            for hh in range(2):
                h = 2 * j + hh
                op = ptile([128, F], "op", 4)
                for vh in range(2):
                    nc.tensor.matmul(
                        op[0 : DH + 1, :],
                        vva[vh][:, ds((DH + 1) * h, DH + 1)],
                        exps[hh][:, ts(vh, F)],
                        start=(vh == 0), stop=(vh == 1),
                    )
                rc = dp.tile([1, F], WT, name="rc", tag=f"rc{hh}", bufs=3)
                with nc.allow_low_precision(reason="softmax recip to bf16"):
                    nc.vector.reciprocal(rc[:, :], op[DH : DH + 1, :])
                ops.append(op)
                rcs.append(rc)
            rcb = ptile([128, F], "op", 4)
            return ops, rcs, rcb

        def emit_tail_b(j, ops_rc):
            ops, rcs, rcb = ops_rc
            ot = dp.tile([128, F], WT, name="ot", tag="ot", bufs=14)
            for hh in range(2):
                nc.tensor.matmul(
                    rcb[ts(hh, DH), :], onesb[:, :], rcs[hh][:, :],
                    start=True, stop=True, tile_position=(0, DH * hh),
                )
            # hw allows only one PSUM operand per DVE op: stage rcb in SBUF
            rcs_b = dp.tile([128, F], WT, name="rcsb", tag="rcsb", bufs=3)
            if j % 2 == 0:
                nc.scalar.activation(rcs_b[:, :], rcb[:, :], AF.Identity)
            else:
                nc.vector.tensor_copy(rcs_b[:, :], rcb[:, :])
            for hh in range(2):
                nc.vector.tensor_tensor(
                    ot[DH * hh : DH * hh + DH, :], ops[hh][0:DH, :],
                    rcs_b[DH * hh : DH * hh + DH, :], ALU.mult,
                )
            return ot

        def emit_xout_m(g, ots, m):
            ps = ptile([128, F], "op", 4)
            for k in range(KH):
                nc.tensor.matmul(
                    ps[:, :], wout[k][:, ts(m, 128)], ots[k][:, :],
                    start=(k == 0), stop=(k == KH - 1),
                )
            xo2 = dp.tile([128, F], BF16, name="xout", tag="xout", bufs=3)
            if m % 2 == 0:
                nc.scalar.activation(
                    xo2[:, :], ps[:, :], AF.Identity, bias=boute[m][:, :],
                    scale=1.0,
                )
            else:
                nc.vector.tensor_scalar_add(xo2[:, :], ps[:, :], boute[m][:, :])
            nc.gpsimd.dma_start(oxo_d[g, ts(m, 128), :], xo2[:, :])

        # ---------------- interleaved software-pipelined graph loop --------
        # Per graph g, the attention pairs are interleaved with "filler"
        # chunks: the front matmuls of graph g+1 and the out-projection of
        # graph g-1.  PE then always has independent work while DVE/ACT
        # digest the recip/exp/normalize chain of the current pair.
        qts_of, ots_of = {}, {}

        def make_front_chunks(g, xts):
            qts = []
            qts_of[g] = qts

            def first():
                emit_fixups(g, xts)
                qts.append(emit_front_q(g, xts, 0))

            chunks = [first]
            for m in range(1, KH):
                chunks.append(lambda m=m: qts.append(emit_front_q(g, xts, m)))
            for m in range(KH):
                chunks.append(lambda m=m: emit_front_h(g, xts, m))
            return chunks

        def make_xout_chunks(g):
            return [lambda m=m: emit_xout_m(g, ots_of[g], m) for m in range(KH)]

        xts0 = emit_xt_dma(0)
        for c in make_front_chunks(0, xts0):
            c()
        for g in range(G):
            qts = qts_of[g]
            ots = [None] * NPAIR
            ots_of[g] = ots
            filler = []
            if g + 1 < G:
                xts_n = emit_xt_dma(g + 1)
                filler += make_front_chunks(g + 1, xts_n)
            if g >= 1:
                filler += make_xout_chunks(g - 1)
            fi = 0

            def fill(n):
                nonlocal fi
                for _ in range(n):
                    if fi < len(filler):
                        filler[fi]()
                        fi += 1

            exps = [None] * NPAIR
            tails = [None] * NPAIR
            exps[0] = emit_scores(0, qts)
            fill(2)
            for j in range(1, NPAIR):
                tails[j - 1] = emit_tail_a(j - 1, exps[j - 1])
                fill(1)
                exps[j] = emit_scores(j, qts)
                ots[j - 1] = emit_tail_b(j - 1, tails[j - 1])
                fill(2)
            tails[NPAIR - 1] = emit_tail_a(NPAIR - 1, exps[NPAIR - 1])
            fill(1)
            ots[NPAIR - 1] = emit_tail_b(NPAIR - 1, tails[NPAIR - 1])
            while fi < len(filler):
                filler[fi]()
                fi += 1
        for c in make_xout_chunks(G - 1):
            c()

    return nc


def _split_multi_waits(json_bytes):
    """Hoist extra sync waits into standalone EventSemaphore instructions.

    This walrus build encodes at most one (wait, update) pair per TPB
    instruction; Tile emits multi-entry on_wait lists, which fail codegen
    with "Too many sync wait commands". Keeping one wait inline and issuing
    the rest as same-engine EventSemaphore instructions immediately before
    is semantically identical (per-engine program order is preserved).
    """
    import orjson

    d = orjson.loads(json_bytes)
    n = 0
    for fn in d["functions"]:
        for blk in fn["blocks"]:
            out = []
            for inst in blk["instructions"]:
                sync = inst.get("sync_info")
                waits = (sync or {}).get("on_wait") or []
                if len(waits) > 1:
                    for w in waits[:-1]:
                        n += 1
                        out.append({
                            "debug": inst.get("debug", 0),
                            "engine": inst["engine"],
                            "ins": [],
                            "name": f"eswait_{n}_{inst['name']}",
                            "opcode": "EventSemaphore",
                            "outs": [],
                            "sync_info": {"on_update": [], "on_wait": [w]},
                        })
                    sync["on_wait"] = [waits[-1]]
                out.append(inst)
            blk["instructions"] = out
    return orjson.dumps(d)


_NC_CACHE = {}


def _get_nc(bias_fix=False):
    if bias_fix not in _NC_CACHE:
        nc = build_program(bias_fix=bias_fix)
        orig = nc.to_json_bytes
        nc.to_json_bytes = lambda: _split_multi_waits(orig())
        _NC_CACHE[bias_fix] = nc
    return _NC_CACHE[bias_fix]


def make_in_maps(x, word_embedding, W_lin, b_lin, W_gcn, b_gcn,
                 in_proj_w, in_proj_b, out_proj_w, out_proj_b):
    f32 = lambda a: np.ascontiguousarray(np.asarray(a), dtype=np.float32)
    wt = lambda a: np.ascontiguousarray(np.asarray(a, dtype=np.float32)).astype(WT_NP)
    x = f32(x)
    WE = f32(word_embedding)
    W_lin, W_gcn, Wout = f32(W_lin), f32(W_gcn), f32(out_proj_w)
    b_lin, b_gcn, b_out = f32(b_lin), f32(b_gcn), f32(out_proj_b)
    ipw, ipb = np.asarray(in_proj_w), np.asarray(in_proj_b)
    Wq, Wk, Wv = (f32(ipw[i * H : (i + 1) * H]) for i in range(3))
    bq, bk, bv = (f32(ipb[i * H : (i + 1) * H]) for i in range(3))

    Wcomb = W_gcn @ W_lin                       # [H, D]
    Wqcomb = Wq @ Wcomb                         # [H, D]
    ktm = Wk @ WE + bk[:, None]                 # [H, V]
    vvh = WE.T @ Wv.T                           # [V, H]
    vva = np.ones((V, NH * (DH + 1)), np.float32)
    for h in range(NH):
        vva[:, (DH + 1) * h : (DH + 1) * h + DH] = vvh[:, DH * h : DH * (h + 1)]

    gb = W_gcn @ b_lin                          # agg-uniform part of b_lin
    bxtf = gb + b_gcn
    bqc = Wq @ bxtf + bq
    boute = Wout @ bv + b_out
    cb1 = C1 * gb
    cq1 = Wq @ cb1
    bias_fix = bool(np.any(b_lin))

    xT = x.reshape(NCORES, G, F, D).transpose(0, 1, 3, 2)  # [cores, G, D, F]
    shared = dict(
        w_qcomb=wt(Wqcomb.T),
        w_comb=wt(Wcomb.T),
        w_out_t=wt(Wout.T),
        k_t=wt(ktm),
        vvh_a=wt(vva),
        ones_r=np.ones((1, DH), np.float32).astype(WT_NP),
        b_qc=bqc.reshape(H, 1),
        b_xt=bxtf.reshape(H, 1),
        b_oute=boute.reshape(H, 1),
    )
    if bias_fix:
        shared["c_b1"] = cb1.reshape(H, 1)
        shared["c_q1"] = cq1.reshape(H, 1)
    return bias_fix, [
        dict(shared, xt=np.ascontiguousarray(xT[c]).astype(WT_NP))
        for c in range(NCORES)
    ]


def gather_outputs(results):
    xt = np.concatenate(
        [np.asarray(r["out_xt"]).astype(np.float32).transpose(0, 2, 1)
         for r in results], axis=0
    )
    xo = np.concatenate(
        [np.asarray(r["out_xo"]).astype(np.float32).transpose(0, 2, 1)
         for r in results], axis=0
    )
    return np.ascontiguousarray(xt), np.ascontiguousarray(xo)


def kernel(**inputs):
    from concourse.bass_utils import run_bass_kernel_spmd

    bias_fix, in_maps = make_in_maps(**inputs)
    nc = _get_nc(bias_fix)
    res = run_bass_kernel_spmd(nc, in_maps, list(range(NCORES)))
    return gather_outputs(res.results)


# revision 38
# speedup vs baseline: 1.0293x; 1.0293x over previous
"""Trainium2 Bass kernel for nn_Encoder_trace (GNN message passing + cross-attention).

Data-parallel over the batch axis B=64 across 8 NeuronCores (8 graphs/core).
Device layout: channels on SBUF partitions, tokens on the free dimension
(everything computed transposed; host un-transposes on gather).

v2: all weight-combination products are precomputed on the HOST (numpy), so
the device kernel is just the per-graph streaming work.  All matmul operands
are bf16.  The chain-GCN aggregation is applied once to xT (it commutes with
every row-mix), the softmax denominators are produced by a ones-column folded
into the vvh stationary, and the reciprocal rows are broadcast across
partitions with a tiny [2,128] selection matmul.

Math per graph (g):
  xa      = agg(xT)                                  (DVE fixup on cols 0..4)
  x_timeT = W_comb @ xa + bxtf                       (W_comb = W_gcn W_lin)
  qT      = W_qcomb @ xa + bqc                       (W_qcomb = Wq W_gcn W_lin)
  kT      = Wk @ WE + bk                 [host]
  vvh     = WE.T @ Wv.T (ones col interleaved per head)  [host]
  per head: scoresT = kT_h.T @ qT_h ; exp (ACT, scale=1/8, no max-sub)
            op[0:65] = [vvh_h | 1].T @ exp           (row 64 = softmax sums)
            rc = recip(op[64]) ; rcb = sel-matmul broadcast of rc
            oT = op[0:64] * rcb
  x_outT  = W_out @ oT + boute                       (boute = W_out bv + b_out)
"""

import numpy as np
from contextlib import ExitStack

import concourse.bass as bass
import concourse.mybir as mybir
import concourse.tile as tile
from concourse.bass import ts, ds

# problem dims (hardcoded per spec)
B, F, D, H, NH, DH, V = 64, 512, 256, 768, 12, 64, 256
NCORES = 8
G = B // NCORES       # graphs per core
KH = H // 128         # 6  (H in 128-partition tiles)
KD = D // 128         # 2  (D in 128-partition tiles)
NPAIR = NH // 2       # 6  head pairs

F32 = mybir.dt.float32
BF16 = mybir.dt.bfloat16
AF = mybir.ActivationFunctionType
ALU = mybir.AluOpType

WT = BF16
WT_NP = mybir.dt.np(WT)

RSQRT2 = float(2.0 ** -0.5)
C1 = RSQRT2 - 0.5  # chain-GCN col-1 colsum deviation (bias correction coeff)


def build_program(bias_fix=False):
    nc = bass.Bass()

    xt_d = nc.declare_dram_parameter("xt", [G, D, F], WT, isOutput=False)
    wqc_d = nc.declare_dram_parameter("w_qcomb", [D, H], WT, isOutput=False)
    wcb_d = nc.declare_dram_parameter("w_comb", [D, H], WT, isOutput=False)
    wot_d = nc.declare_dram_parameter("w_out_t", [H, H], WT, isOutput=False)
    kt_d = nc.declare_dram_parameter("k_t", [H, V], WT, isOutput=False)
    vva_d = nc.declare_dram_parameter("vvh_a", [D, NH * (DH + 1)], WT, isOutput=False)
    bqc_d = nc.declare_dram_parameter("b_qc", [H, 1], F32, isOutput=False)
    bxt_d = nc.declare_dram_parameter("b_xt", [H, 1], F32, isOutput=False)
    bou_d = nc.declare_dram_parameter("b_oute", [H, 1], F32, isOutput=False)
    ones_d = nc.declare_dram_parameter("ones_r", [1, DH], WT, isOutput=False)
    if bias_fix:
        cb1_d = nc.declare_dram_parameter("c_b1", [H, 1], F32, isOutput=False)
        cq1_d = nc.declare_dram_parameter("c_q1", [H, 1], F32, isOutput=False)
    oxt_d = nc.declare_dram_parameter("out_xt", [G, H, F], BF16, isOutput=True)
    oxo_d = nc.declare_dram_parameter("out_xo", [G, H, F], BF16, isOutput=True)

    with ExitStack() as ctx:
        tc = ctx.enter_context(tile.TileContext(nc))
        wp = ctx.enter_context(tc.tile_pool(name="wp", bufs=1))
        pp = ctx.enter_context(tc.tile_pool(name="pp", bufs=1, space="PSUM"))
        dp = ctx.enter_context(tc.tile_pool(name="dp", bufs=1))

        def ptile(shape, tag, bufs):
            return pp.tile(shape, F32, name=tag, tag=tag, bufs=bufs)

        def wtile(shape, dt, tag):
            return wp.tile(shape, dt, name=tag, tag=tag)


        # ---------------- persistent weights (DMA in consumer order) -------
        wqc = [wtile([128, H], WT, f"wqc{k}") for k in range(KD)]
        wcb = [wtile([128, H], WT, f"wcb{k}") for k in range(KD)]
        kt = [wtile([128, V], WT, f"kt{m}") for m in range(KH)]
        vva = [wtile([128, NH * (DH + 1)], WT, f"vva{m}") for m in range(KD)]
        wout = [wtile([128, H], WT, f"wout{k}") for k in range(KH)]
        onesb = wtile([1, DH], WT, "onesb")
        bqc = [wtile([128, 1], F32, f"bqc{m}") for m in range(KH)]
        bxtf = [wtile([128, 1], F32, f"bxtf{m}") for m in range(KH)]
        boute = [wtile([128, 1], F32, f"boute{m}") for m in range(KH)]
        if bias_fix:
            cb1 = [wtile([128, 1], F32, f"cb1{m}") for m in range(KH)]
            cq1 = [wtile([128, 1], F32, f"cq1{m}") for m in range(KH)]

        for k in range(KD):
            nc.gpsimd.dma_start(wqc[k][:, :], wqc_d[ts(k, 128), :])
        for k in range(KD):
            nc.gpsimd.dma_start(wcb[k][:, :], wcb_d[ts(k, 128), :])
        for m in range(KH):
            nc.gpsimd.dma_start(bqc[m][:, :], bqc_d[ts(m, 128), :])
            nc.gpsimd.dma_start(bxtf[m][:, :], bxt_d[ts(m, 128), :])
            if bias_fix:
                nc.gpsimd.dma_start(cb1[m][:, :], cb1_d[ts(m, 128), :])
                nc.gpsimd.dma_start(cq1[m][:, :], cq1_d[ts(m, 128), :])
        for m in range(KH):
            nc.gpsimd.dma_start(kt[m][:, :], kt_d[ts(m, 128), :])
        for m in range(KD):
            nc.gpsimd.dma_start(vva[m][:, :], vva_d[ts(m, 128), :])
        nc.gpsimd.dma_start(onesb[:, :], ones_d[:, :])
        for k in range(KH):
            nc.gpsimd.dma_start(wout[k][:, :], wot_d[ts(k, 128), :])
        for m in range(KH):
            nc.gpsimd.dma_start(boute[m][:, :], bou_d[ts(m, 128), :])

        # ---------------- per-graph emission ----------------
        def emit_xt_dma(g):
            xts = []
            for k in range(KD):
                t = dp.tile([128, F], WT, name="xtin", tag="xtin", bufs=6)
                nc.sync.dma_start(t[:, :], xt_d[g, ts(k, 128), :])
                xts.append(t)
            return xts

        def emit_fixups(g, xts):
            # chain-GCN aggregation applied once to xT (commutes with the
            # row mixes).  Cols: 0: c0; 1: .5c1+r2*c0; 2..4: .5ck+.5c(k-1).
            for t in xts:
                xs = dp.tile([128, 4], WT, name="xsc", tag="xsc", bufs=6)
                nc.vector.tensor_copy(xs[:, :], t[:, 0:4])
                nc.vector.tensor_scalar_mul(t[:, 1:5], t[:, 1:5], 0.5)
                nc.vector.scalar_tensor_tensor(
                    t[:, 2:5], xs[:, 1:4], 0.5, t[:, 2:5], ALU.mult, ALU.add
                )
                nc.vector.scalar_tensor_tensor(
                    t[:, 1:2], xs[:, 0:1], RSQRT2, t[:, 1:2], ALU.mult, ALU.add
                )

        def emit_front_q(g, xts, m):
            ps = ptile([128, F], "op", 4)
            for k in range(KD):
                nc.tensor.matmul(
                    ps[:, :], wqc[k][:, ts(m, 128)], xts[k][:, :],
                    start=(k == 0), stop=(k == KD - 1),
                )
            qt = dp.tile([128, F], WT, name="qt", tag="qt", bufs=12)
            if m % 2 == 0:
                nc.scalar.activation(
                    qt[:, :], ps[:, :], AF.Identity, bias=bqc[m][:, :], scale=1.0
                )
            else:
                nc.vector.tensor_scalar_add(qt[:, :], ps[:, :], bqc[m][:, :])
            if bias_fix:
                nc.vector.scalar_tensor_tensor(
                    qt[:, 1:2], cq1[m][:, :], 1.0, qt[:, 1:2], ALU.mult, ALU.add
                )
            return qt

        def emit_front_h(g, xts, m):
            ps = ptile([128, F], "op", 4)
            for k in range(KD):
                nc.tensor.matmul(
                    ps[:, :], wcb[k][:, ts(m, 128)], xts[k][:, :],
                    start=(k == 0), stop=(k == KD - 1),
                )
            xo = dp.tile([128, F], BF16, name="xtime", tag="xtime", bufs=4)
            nc.scalar.activation(
                xo[:, :], ps[:, :], AF.Identity, bias=bxtf[m][:, :], scale=1.0
            )
            if bias_fix:
                nc.vector.scalar_tensor_tensor(
                    xo[:, 1:2], cb1[m][:, :], 1.0, xo[:, 1:2], ALU.mult, ALU.add
                )
            nc.gpsimd.dma_start(oxt_d[g, ts(m, 128), :], xo[:, :])

        def emit_scores(j, qts):
            exps = []
            for hh in range(2):
                r = DH * hh
                sc = ptile([128, 2 * F], "score", 2)
                for vh in range(2):
                    nc.tensor.matmul(
                        sc[:, ts(vh, F)],
                        kt[j][r : r + DH, ts(vh, 128)],
                        qts[j][r : r + DH, :],
                        start=True, stop=True,
                    )
                ex = dp.tile([128, 2 * F], WT, name="exp", tag="exp", bufs=6)
                nc.scalar.activation(ex[:, :], sc[:, :], AF.Exp, scale=0.125)
                exps.append(ex)
            return exps

        def emit_tail_a(j, exps):
            ops, rcs = [], []
            for hh in range(2):
                h = 2 * j + hh
                op = ptile([128, F], "op", 4)
                for vh in range(2):
                    nc.tensor.matmul(
                        op[0 : DH + 1, :],
                        vva[vh][:, ds((DH + 1) * h, DH + 1)],
                        exps[hh][:, ts(vh, F)],
                        start=(vh == 0), stop=(vh == 1),
                    )
                rc = dp.tile([1, F], WT, name="rc", tag=f"rc{hh}", bufs=4)
                with nc.allow_low_precision(reason="softmax recip to bf16"):
                    nc.vector.reciprocal(rc[:, :], op[DH : DH + 1, :])
                ops.append(op)
                rcs.append(rc)
            rcb = ptile([128, F], "op", 4)
            return ops, rcs, rcb

        def emit_tail_b(j, ops_rc):
            ops, rcs, rcb = ops_rc
            ot = dp.tile([128, F], WT, name="ot", tag="ot", bufs=16)
            for hh in range(2):
                nc.tensor.matmul(
                    rcb[ts(hh, DH), :], onesb[:, :], rcs[hh][:, :],
                    start=True, stop=True, tile_position=(0, DH * hh),
                )
            # hw allows only one PSUM operand per DVE op: stage rcb in SBUF
            rcs_b = dp.tile([128, F], WT, name="rcsb", tag="rcsb", bufs=6)
            if j % 2 == 0:
                nc.scalar.activation(rcs_b[:, :], rcb[:, :], AF.Identity)
            else:
                nc.vector.tensor_copy(rcs_b[:, :], rcb[:, :])
            for hh in range(2):
                nc.vector.tensor_tensor(
                    ot[DH * hh : DH * hh + DH, :], ops[hh][0:DH, :],
                    rcs_b[DH * hh : DH * hh + DH, :], ALU.mult,
                )
            return ot

        def emit_xout_m(g, ots, m):
            ps = ptile([128, F], "op", 4)
            for k in range(KH):
                nc.tensor.matmul(
                    ps[:, :], wout[k][:, ts(m, 128)], ots[k][:, :],
                    start=(k == 0), stop=(k == KH - 1),
                )
            xo2 = dp.tile([128, F], BF16, name="xout", tag="xout", bufs=4)
            if m % 2 == 0:
                nc.scalar.activation(
                    xo2[:, :], ps[:, :], AF.Identity, bias=boute[m][:, :],
                    scale=1.0,
                )
            else:
                nc.vector.tensor_scalar_add(xo2[:, :], ps[:, :], boute[m][:, :])
            nc.gpsimd.dma_start(oxo_d[g, ts(m, 128), :], xo2[:, :])

        # ---------------- interleaved software-pipelined graph loop --------
        # Per graph g, the attention pairs are interleaved with "filler"
        # chunks: the front matmuls of graph g+1 and the out-projection of
        # graph g-1.  PE then always has independent work while DVE/ACT
        # digest the recip/exp/normalize chain of the current pair.
        qts_of, ots_of = {}, {}

        def make_front_chunks(g, xts):
            qts = []
            qts_of[g] = qts

            def first():
                emit_fixups(g, xts)
                qts.append(emit_front_q(g, xts, 0))

            chunks = [first]
            for m in range(1, KH):
                chunks.append(lambda m=m: qts.append(emit_front_q(g, xts, m)))
            for m in range(KH):
                chunks.append(lambda m=m: emit_front_h(g, xts, m))
            return chunks

        def make_xout_chunks(g):
            return [lambda m=m: emit_xout_m(g, ots_of[g], m) for m in range(KH)]

        xts0 = emit_xt_dma(0)
        for c in make_front_chunks(0, xts0):
            c()
        for g in range(G):
            qts = qts_of[g]
            ots = [None] * NPAIR
            ots_of[g] = ots
            filler = []
            if g + 1 < G:
                xts_n = emit_xt_dma(g + 1)
                filler += make_front_chunks(g + 1, xts_n)
            if g >= 1:
                filler += make_xout_chunks(g - 1)
            fi = 0

            def fill(n):
                nonlocal fi
                for _ in range(n):
                    if fi < len(filler):
                        filler[fi]()
                        fi += 1

            exps = [None] * NPAIR
            tails = [None] * NPAIR
            exps[0] = emit_scores(0, qts)
            fill(2)
            for j in range(1, NPAIR):
                tails[j - 1] = emit_tail_a(j - 1, exps[j - 1])
                fill(1)
                exps[j] = emit_scores(j, qts)
                ots[j - 1] = emit_tail_b(j - 1, tails[j - 1])
                fill(2)
            tails[NPAIR - 1] = emit_tail_a(NPAIR - 1, exps[NPAIR - 1])
            fill(1)
            ots[NPAIR - 1] = emit_tail_b(NPAIR - 1, tails[NPAIR - 1])
            while fi < len(filler):
                filler[fi]()
                fi += 1
        for c in make_xout_chunks(G - 1):
            c()

    return nc


def _split_multi_waits(json_bytes):
    """Hoist extra sync waits into standalone EventSemaphore instructions.

    This walrus build encodes at most one (wait, update) pair per TPB
    instruction; Tile emits multi-entry on_wait lists, which fail codegen
    with "Too many sync wait commands". Keeping one wait inline and issuing
    the rest as same-engine EventSemaphore instructions immediately before
    is semantically identical (per-engine program order is preserved).
    """
    import orjson

    d = orjson.loads(json_bytes)
    n = 0
    for fn in d["functions"]:
        for blk in fn["blocks"]:
            out = []
            for inst in blk["instructions"]:
                sync = inst.get("sync_info")
                waits = (sync or {}).get("on_wait") or []
                if len(waits) > 1:
                    for w in waits[:-1]:
                        n += 1
                        out.append({
                            "debug": inst.get("debug", 0),
                            "engine": inst["engine"],
                            "ins": [],
                            "name": f"eswait_{n}_{inst['name']}",
                            "opcode": "EventSemaphore",
                            "outs": [],
                            "sync_info": {"on_update": [], "on_wait": [w]},
                        })
                    sync["on_wait"] = [waits[-1]]
                out.append(inst)
            blk["instructions"] = out
    return orjson.dumps(d)


_NC_CACHE = {}


def _get_nc(bias_fix=False):
    if bias_fix not in _NC_CACHE:
        nc = build_program(bias_fix=bias_fix)
        orig = nc.to_json_bytes
        nc.to_json_bytes = lambda: _split_multi_waits(orig())
        _NC_CACHE[bias_fix] = nc
    return _NC_CACHE[bias_fix]


def make_in_maps(x, word_embedding, W_lin, b_lin, W_gcn, b_gcn,
                 in_proj_w, in_proj_b, out_proj_w, out_proj_b):
    f32 = lambda a: np.ascontiguousarray(np.asarray(a), dtype=np.float32)
    wt = lambda a: np.ascontiguousarray(np.asarray(a, dtype=np.float32)).astype(WT_NP)
    x = f32(x)
    WE = f32(word_embedding)
    W_lin, W_gcn, Wout = f32(W_lin), f32(W_gcn), f32(out_proj_w)
    b_lin, b_gcn, b_out = f32(b_lin), f32(b_gcn), f32(out_proj_b)
    ipw, ipb = np.asarray(in_proj_w), np.asarray(in_proj_b)
    Wq, Wk, Wv = (f32(ipw[i * H : (i + 1) * H]) for i in range(3))
    bq, bk, bv = (f32(ipb[i * H : (i + 1) * H]) for i in range(3))

    Wcomb = W_gcn @ W_lin                       # [H, D]
    Wqcomb = Wq @ Wcomb                         # [H, D]
    ktm = Wk @ WE + bk[:, None]                 # [H, V]
    vvh = WE.T @ Wv.T                           # [V, H]
    vva = np.ones((V, NH * (DH + 1)), np.float32)
    for h in range(NH):
        vva[:, (DH + 1) * h : (DH + 1) * h + DH] = vvh[:, DH * h : DH * (h + 1)]

    gb = W_gcn @ b_lin                          # agg-uniform part of b_lin
    bxtf = gb + b_gcn
    bqc = Wq @ bxtf + bq
    boute = Wout @ bv + b_out
    cb1 = C1 * gb
    cq1 = Wq @ cb1
    bias_fix = bool(np.any(b_lin))

    xT = x.reshape(NCORES, G, F, D).transpose(0, 1, 3, 2)  # [cores, G, D, F]
    shared = dict(
        w_qcomb=wt(Wqcomb.T),
        w_comb=wt(Wcomb.T),
        w_out_t=wt(Wout.T),
        k_t=wt(ktm),
        vvh_a=wt(vva),
        ones_r=np.ones((1, DH), np.float32).astype(WT_NP),
        b_qc=bqc.reshape(H, 1),
        b_xt=bxtf.reshape(H, 1),
        b_oute=boute.reshape(H, 1),
    )
    if bias_fix:
        shared["c_b1"] = cb1.reshape(H, 1)
        shared["c_q1"] = cq1.reshape(H, 1)
    return bias_fix, [
        dict(shared, xt=np.ascontiguousarray(xT[c]).astype(WT_NP))
        for c in range(NCORES)
    ]


def gather_outputs(results):
    xt = np.concatenate(
        [np.asarray(r["out_xt"]).astype(np.float32).transpose(0, 2, 1)
         for r in results], axis=0
    )
    xo = np.concatenate(
        [np.asarray(r["out_xo"]).astype(np.float32).transpose(0, 2, 1)
         for r in results], axis=0
    )
    return np.ascontiguousarray(xt), np.ascontiguousarray(xo)


def kernel(**inputs):
    from concourse.bass_utils import run_bass_kernel_spmd

    bias_fix, in_maps = make_in_maps(**inputs)
    nc = _get_nc(bias_fix)
    res = run_bass_kernel_spmd(nc, in_maps, list(range(NCORES)))
    return gather_outputs(res.results)
